# revision 14
# baseline (speedup 1.0000x reference)
"""Trainium2 Bass kernel for nn_ExampleModel_9234179686517 (dense_mlp).

Model: bilinear grid-sample of a (4, 512, 512) featuremap at 4M points,
concat with xyz, then a 7->16->16->16->16->3 ReLU MLP.

Strategy (pure data parallel over 8 NeuronCores):
 - Host precomputes a bf16 "window table" Qb[y*64+xb] = pair-rows y,y+1
   (border-clamped) for the 16 x-positions [8*xb, 8*xb+16), all 4
   channels: 2*16*4 bf16 = 256B per row, 32768 rows (15-bit int16 idx).
 - The main point-major pipeline computes the gather index; it is
   bounced through DRAM into the wrapped-16 idx layout dma_gather
   requires (replicated to all 8 Q7 core groups) and permuted on DVE.
 - Gathers run on 4 SWDGE queues round-robin so Q7 descriptor
   generation overlaps the per-queue DMA transfers (the single-queue
   ring otherwise serializes desc-gen with the previous transfer).
 - The 3-bit sub-window x-position is resolved with conditional shifted
   copies on DVE: plain bf16 copies + predicated copies on fp32-bitcast
   views (copy_predicated has no fast mode, so halving the element
   count halves its cost; masks must be an integer dtype).
 - TensorE transposes point-major -> feature-major and runs the MLP as
   block-diagonal (8 networks wide) bf16 matmuls with fp32 PSUM
   accumulation; ReLU+bias on ScalarE/VectorE; results un-transposed on
   TensorE and DMAed back.
"""

import sys

for _p in ("/opt/trn_rl_repo", "/root/.axon_site/_ro/trn_rl_repo"):
    if _p not in sys.path:
        sys.path.insert(0, _p)

import numpy as np
import ml_dtypes

BF16 = ml_dtypes.bfloat16

N_TOTAL = 4_000_000
N_CORES = 8
C, H, W = 4, 512, 512
HID = 16

P = 128          # partitions
S = 512          # slots per lane per coord tile
GS = 64          # slots per lane per MLP group (8192 points)
FPAD = 8         # padded feature count (7 real + 1 zero)
TCH = 64         # slots per gather chunk (8192 points)

N_CORE = N_TOTAL // N_CORES               # 500_000
M_SLOTS = 4096                            # slots per lane (multiple of S)
N_PAD = P * M_SLOTS                       # 524_288 padded points per core

NROWS = 512 * 64                          # window-table rows (= 32768)
NQUEUES = 4                               # SWDGE queues for gathers
import os as _os
INPLACE_TREE = _os.environ.get("INPLACE_TREE", "0") == "1"
FUSED_RELU = _os.environ.get("FUSED_RELU", "1") == "1"


def _build_host_constants(featuremap, Ws, bs):
    """Window table + block-diagonal bf16 weights."""
    fmT = np.ascontiguousarray(featuremap.transpose(1, 2, 0)).astype(np.float32)
    # pair-row, x-window layout: row (y, xb) -> [r(2), s(16), c(4)]
    ys = np.arange(H)
    y2 = np.stack([ys, np.minimum(ys + 1, H - 1)], 1)            # [512, 2]
    xs = (np.arange(64)[:, None] * 8 + np.arange(16)[None, :])   # [64, 16]
    xs = np.minimum(xs, W - 1)
    # qtab[y, xb, r, s, c]
    qtab = fmT[y2[:, None, :, None], xs[None, :, None, :], :]    # [512, 64, 2, 16, 4]
    qtab = qtab.reshape(NROWS, 128).astype(BF16)

    W1, W2, W3, W4, W5 = Ws
    b1, b2, b3, b4, b5 = bs

    W1a = np.zeros((FPAD, HID), np.float32)
    W1a[:7] = W1

    def blockdiag(Wm, nb):
        fi, fo = Wm.shape
        out = np.zeros((fi * nb, fo * nb), np.float32)
        for b in range(nb):
            out[b * fi:(b + 1) * fi, b * fo:(b + 1) * fo] = Wm
        return out

    w1blk = blockdiag(W1a, 8)                      # [64, 128]
    w1stack = np.concatenate([w1blk, w1blk], 0)    # [128, 128]

    return {
        "qtab": qtab,
        "w1stack": w1stack.astype(BF16),
        "w2blk": blockdiag(W2, 8).astype(BF16),
        "w3blk": blockdiag(W3, 8).astype(BF16),
        "w4blk": blockdiag(W4, 8).astype(BF16),
        "w5blk": blockdiag(W5, 8).astype(BF16),
        "b1blk": np.tile(b1, 8).reshape(P, 1).astype(np.float32),
        "b2blk": np.tile(b2, 8).reshape(P, 1).astype(np.float32),
        "b3blk": np.tile(b3, 8).reshape(P, 1).astype(np.float32),
        "b4blk": np.tile(b4, 8).reshape(P, 1).astype(np.float32),
        "b5blk": np.tile(b5, 8).reshape(24, 1).astype(np.float32),
        "id128": np.eye(P, dtype=np.float32).astype(BF16),
        "id24": np.eye(24, dtype=np.float32),
    }


def build_program(n_slots=M_SLOTS, s_tile=S, mlp=True, gather=True):
    """Build the per-core Bass program (same program for all 8 cores)."""
    import concourse.bass as bass
    import concourse.tile as tile
    from concourse import bacc, mybir

    f32 = mybir.dt.float32
    bf16 = mybir.dt.bfloat16
    i16 = mybir.dt.int16
    AF = mybir.ActivationFunctionType
    OP = mybir.AluOpType

    assert n_slots % s_tile == 0 and s_tile % GS == 0 and s_tile % TCH == 0
    n_pad = P * n_slots
    n_iters = n_slots // s_tile
    chunks = s_tile // TCH           # gather chunks per coord tile
    GNI = 1024                       # idxs per dma_gather call (ucode limit)
    gcalls = P * TCH // GNI          # gather calls per chunk (8)
    TW = P * TCH // 16               # wrapped idx columns per chunk (512)

    nc = bacc.Bacc("TRN2", target_bir_lowering=False, debug=False,
                   enable_asserts=False, num_devices=N_CORES,
                   num_swdge_queues=NQUEUES,
                   dynamic_dma_scratch_size=32768)

    xin = nc.dram_tensor("x", [n_pad, 3], f32, kind="ExternalInput").ap()
    qtab = nc.dram_tensor("qtab", [NROWS, 128], bf16, kind="ExternalInput").ap()
    w1stack = nc.dram_tensor("w1stack", [P, P], bf16, kind="ExternalInput").ap()
    w2 = nc.dram_tensor("w2blk", [P, P], bf16, kind="ExternalInput").ap()
    w3 = nc.dram_tensor("w3blk", [P, P], bf16, kind="ExternalInput").ap()
    w4 = nc.dram_tensor("w4blk", [P, P], bf16, kind="ExternalInput").ap()
    w5 = nc.dram_tensor("w5blk", [P, 24], bf16, kind="ExternalInput").ap()
    b1 = nc.dram_tensor("b1blk", [P, 1], f32, kind="ExternalInput").ap()
    b2i = nc.dram_tensor("b2blk", [P, 1], f32, kind="ExternalInput").ap()
    b3i = nc.dram_tensor("b3blk", [P, 1], f32, kind="ExternalInput").ap()
    b4i = nc.dram_tensor("b4blk", [P, 1], f32, kind="ExternalInput").ap()
    b5i = nc.dram_tensor("b5blk", [24, 1], f32, kind="ExternalInput").ap()
    id128 = nc.dram_tensor("id128", [P, P], bf16, kind="ExternalInput").ap()
    id24 = nc.dram_tensor("id24", [24, 24], f32, kind="ExternalInput").ap()
    yout = nc.dram_tensor("y", [n_pad, 3], f32, kind="ExternalOutput").ap()

    # lane p owns rows [p*n_slots, (p+1)*n_slots)  (contiguous HBM runs)
    xv = xin.rearrange("(p s) c -> p s c", p=P)
    yv = yout.rearrange("(p s) c -> p s c", p=P)

    BIGF = float(2 ** 23)

    from contextlib import ExitStack

    with tile.TileContext(nc) as tc, ExitStack() as ctx:
            ep = ctx.enter_context
            consts = ep(tc.tile_pool(name="consts", bufs=1))
            xio = ep(tc.tile_pool(name="xio", bufs=2))
            coord2 = ep(tc.tile_pool(name="coord2", bufs=2))
            coord1 = ep(tc.tile_pool(name="coord1", bufs=1))
            jdp = ep(tc.tile_pool(name="jd", bufs=4, space="DRAM"))
            jwp = ep(tc.tile_pool(name="jw", bufs=4))
            jtp = ep(tc.tile_pool(name="jt", bufs=4))
            gatp = ep(tc.tile_pool(name="gat", bufs=3))
            shiftp = ep(tc.tile_pool(name="shift", bufs=1))
            stagep = ep(tc.tile_pool(name="stage", bufs=2))
            tsbp = ep(tc.tile_pool(name="tsb", bufs=3))
            actsp = ep(tc.tile_pool(name="acts", bufs=4))
            s5p = ep(tc.tile_pool(name="s5", bufs=2))
            ostagep = ep(tc.tile_pool(name="ostage", bufs=2))
            ptr = ep(tc.tile_pool(name="ptr", bufs=2, space="PSUM"))
            pmm = ep(tc.tile_pool(name="pmm", bufs=2, space="PSUM"))
            p5 = ep(tc.tile_pool(name="p5", bufs=1, space="PSUM"))

            # ---- constants into SBUF
            w1_sb = consts.tile([P, P], bf16, tag="w1")
            w2_sb = consts.tile([P, P], bf16, tag="w2")
            w3_sb = consts.tile([P, P], bf16, tag="w3")
            w4_sb = consts.tile([P, P], bf16, tag="w4")
            w5_sb = consts.tile([P, 24], bf16, tag="w5")
            b1_sb = consts.tile([P, 1], f32, tag="b1")
            b2_sb = consts.tile([P, 1], f32, tag="b2")
            b3_sb = consts.tile([P, 1], f32, tag="b3")
            b4_sb = consts.tile([P, 1], f32, tag="b4")
            b5_sb = consts.tile([24, 1], f32, tag="b5")
            id128_sb = consts.tile([P, P], bf16, tag="id128")
            id24_sb = consts.tile([24, 24], f32, tag="id24")
            cm05 = consts.tile([P, 1], f32, tag="cm05")
            nc.vector.memset(cm05[:], -0.5)
            for sb, src in (
                (w1_sb, w1stack), (w2_sb, w2), (w3_sb, w3), (w4_sb, w4),
                (w5_sb, w5), (b1_sb, b1), (b2_sb, b2i), (b3_sb, b3i),
                (b4_sb, b4i), (b5_sb, b5i), (id128_sb, id128), (id24_sb, id24),
            ):
                nc.sync.dma_start(out=sb[:], in_=src)

            def floor1(pool, fsrc, tagp):
                """round(x - 0.5): == floor(x) for non-integral x >= 0; for
                integral x may yield x-1, which downstream lerps absorb
                (weight saturates to 1.0 on the correct sample)."""
                b_ = pool.tile([P, fsrc.shape[1]], f32, tag=f"fl{tagp}")
                nc.vector.tensor_scalar(out=b_[:], in0=fsrc,
                                        scalar1=BIGF - 0.5, scalar2=BIGF,
                                        op0=OP.add, op1=OP.subtract)
                return b_

            def run_mlp(stg, sl0):
                ost = ostagep.tile([P, s_tile, 3], f32, tag="ost")
                stg_flat = stg.rearrange("p s f -> p (s f)")
                groups = s_tile // GS
                for g in range(groups):
                    t_ps = ptr.tile([P, 4, P], bf16, tag="tp")
                    for c4 in range(4):
                        base = (g * GS + c4 * 16) * FPAD
                        nc.tensor.transpose(out=t_ps[:, c4, :],
                                            in_=stg_flat[:, base:base + P],
                                            identity=id128_sb[:])
                    t_sb = tsbp.tile([P, 4, P], bf16, tag="tsb")
                    nc.scalar.activation(out=t_sb[:], in_=t_ps[:],
                                         func=AF.Copy, bias=0.0, scale=1.0)

                    ps = pmm.tile([P, 1024], f32, tag="ps")
                    for c4 in range(4):
                        nc.tensor.matmul(out=ps[:, c4 * P:(c4 + 1) * P],
                                         lhsT=w1_sb[0:64, :], rhs=t_sb[0:64, c4, :],
                                         start=True, stop=True)
                        nc.tensor.matmul(out=ps[:, 512 + c4 * P:512 + (c4 + 1) * P],
                                         lhsT=w1_sb[64:128, :], rhs=t_sb[64:128, c4, :],
                                         start=True, stop=True)
                    h = actsp.tile([P, 1024], bf16, tag="h")
                    if FUSED_RELU:
                        nc.scalar.activation(out=h[:], in_=ps[:],
                                             func=AF.Relu, bias=b1_sb[:], scale=1.0)
                    else:
                        nc.scalar.activation(out=h[:, 0:512], in_=ps[:, 0:512],
                                             func=AF.Relu, bias=b1_sb[:], scale=1.0)
                        nc.scalar.activation(out=h[:, 512:1024], in_=ps[:, 512:1024],
                                             func=AF.Relu, bias=b1_sb[:], scale=1.0)

                    for w_sb, bias_sb in ((w2_sb, b2_sb), (w3_sb, b3_sb), (w4_sb, b4_sb)):
                        ps = pmm.tile([P, 1024], f32, tag="ps")
                        nc.tensor.matmul(out=ps[:, 0:512], lhsT=w_sb[:], rhs=h[:, 0:512],
                                         start=True, stop=True)
                        nc.tensor.matmul(out=ps[:, 512:1024], lhsT=w_sb[:],
                                         rhs=h[:, 512:1024], start=True, stop=True)
                        h = actsp.tile([P, 1024], bf16, tag="h")
                        if FUSED_RELU:
                            nc.scalar.activation(out=h[:], in_=ps[:],
                                                 func=AF.Relu, bias=bias_sb[:], scale=1.0)
                        else:
                            nc.scalar.activation(out=h[:, 0:512], in_=ps[:, 0:512],
                                                 func=AF.Relu, bias=bias_sb[:], scale=1.0)
                            nc.scalar.activation(out=h[:, 512:1024], in_=ps[:, 512:1024],
                                                 func=AF.Relu, bias=bias_sb[:], scale=1.0)

                    ps5 = p5.tile([24, 1024], f32, tag="ps5")
                    nc.tensor.matmul(out=ps5[:, 0:512], lhsT=w5_sb[:], rhs=h[:, 0:512],
                                     start=True, stop=True)
                    nc.tensor.matmul(out=ps5[:, 512:1024], lhsT=w5_sb[:],
                                     rhs=h[:, 512:1024], start=True, stop=True)
                    s5 = s5p.tile([24, 1024], f32, tag="s5")
                    nc.scalar.activation(out=s5[:], in_=ps5[:], func=AF.Identity,
                                         bias=b5_sb[:], scale=1.0)

                    u_ps = ptr.tile([P, 8, 24], f32, tag="tp")
                    for ui in range(2):
                        for c4 in range(4):
                            nc.tensor.transpose(
                                out=u_ps[:, c4 * 2 + ui, :],
                                in_=s5[:, ui * 512 + c4 * P: ui * 512 + (c4 + 1) * P],
                                identity=id24_sb[:])
                    uv = u_ps.rearrange("p k (b c) -> p k b c", c=3)
                    ostg = ost[:, g * GS:(g + 1) * GS, :].rearrange(
                        "p (c u b) d -> p c u b d", c=4, u=2)
                    for ui in range(2):
                        nc.vector.tensor_copy(out=ostg[:, :, ui, :, :],
                                              in_=uv[:, ui::2, :, :])

                nc.sync.dma_start(out=yv[:, sl0:sl0 + s_tile, :], in_=ost[:])

            pending_mlp = None

            for it in range(n_iters):
                sl0 = it * s_tile

                # ======== point-major coordinate pipeline ========
                xt = xio.tile([P, s_tile, 3], f32, tag="xt")
                nc.sync.dma_start(out=xt[:], in_=xv[:, sl0:sl0 + s_tile, :])

                fx = coord2.tile([P, s_tile], f32, tag="fx")
                nc.scalar.activation(out=fx[:], in_=xt[:, :, 0], func=AF.Relu,
                                     bias=cm05[:], scale=float(W))
                fy = coord2.tile([P, s_tile], f32, tag="fy")
                nc.scalar.activation(out=fy[:], in_=xt[:, :, 1], func=AF.Relu,
                                     bias=cm05[:], scale=float(H))

                u8 = coord1.tile([P, s_tile], f32, tag="u8")
                nc.vector.tensor_scalar(out=u8[:], in0=fx[:], scalar1=0.125,
                                        scalar2=None, op0=OP.mult)
                xbf = floor1(coord1, u8[:], "x")
                u = coord1.tile([P, s_tile], f32, tag="u")
                nc.vector.scalar_tensor_tensor(out=u[:], in0=xbf[:], scalar=-8.0,
                                               in1=fx[:], op0=OP.mult, op1=OP.add)
                # sub-window bit masks (f32 0/1) + wx
                b2f = coord2.tile([P, s_tile], f32, tag="b2f")
                nc.vector.tensor_scalar(out=b2f[:], in0=u[:], scalar1=4.0,
                                        scalar2=None, op0=OP.is_ge)
                u2 = coord1.tile([P, s_tile], f32, tag="u2")
                nc.vector.scalar_tensor_tensor(out=u2[:], in0=b2f[:], scalar=-4.0,
                                               in1=u[:], op0=OP.mult, op1=OP.add)
                b1f = coord2.tile([P, s_tile], f32, tag="b1f")
                nc.vector.tensor_scalar(out=b1f[:], in0=u2[:], scalar1=2.0,
                                        scalar2=None, op0=OP.is_ge)
                u3 = coord1.tile([P, s_tile], f32, tag="u3")
                nc.vector.scalar_tensor_tensor(out=u3[:], in0=b1f[:], scalar=-2.0,
                                               in1=u2[:], op0=OP.mult, op1=OP.add)
                b0f = coord2.tile([P, s_tile], f32, tag="b0f")
                nc.vector.tensor_scalar(out=b0f[:], in0=u3[:], scalar1=1.0,
                                        scalar2=None, op0=OP.is_ge)
                wx = coord2.tile([P, s_tile], bf16, tag="wx")
                nc.vector.tensor_tensor(out=wx[:], in0=u3[:], in1=b0f[:], op=OP.subtract)
                # integer copies for copy_predicated masks (verifier requires
                # an integer mask dtype)
                u8d = mybir.dt.uint8
                mb2 = coord2.tile([P, s_tile], u8d, tag="mb2")
                nc.vector.tensor_copy(out=mb2[:], in_=b2f[:])
                mb1 = coord2.tile([P, s_tile], u8d, tag="mb1")
                nc.vector.tensor_copy(out=mb1[:], in_=b1f[:])
                mb0 = coord2.tile([P, s_tile], u8d, tag="mb0")
                nc.vector.tensor_copy(out=mb0[:], in_=b0f[:])

                iyf = floor1(coord1, fy[:], "y")
                wy = coord2.tile([P, s_tile], bf16, tag="wy")
                nc.vector.tensor_tensor(out=wy[:], in0=fy[:], in1=iyf[:], op=OP.subtract)

                # gather row index (int16), still point-major
                idxf = coord1.tile([P, s_tile], f32, tag="idxf")
                nc.vector.scalar_tensor_tensor(out=idxf[:], in0=iyf[:], scalar=64.0,
                                               in1=xbf[:], op0=OP.mult, op1=OP.add)
                idx16 = coord2.tile([P, s_tile], i16, tag="idx16")
                nc.vector.tensor_copy(out=idx16[:], in_=idxf[:])

                # ======== staging + per-chunk gather/select/lerp ========
                stg = stagep.tile([P, s_tile, FPAD], bf16, tag="stg")
                nc.vector.memset(stg[:, :, 7], 0.0)
                nc.scalar.activation(out=stg[:, :, 0:3], in_=xt[:],
                                     func=AF.Copy, bias=0.0, scale=1.0)

                for ch in range(chunks):
                    cs = ch * TCH
                    # --- idx to wrapped-16 layout via DRAM bounce ---
                    # DRAM layout [q, tl, s]: q = p%16, tl = p//16
                    jd = jdp.tile([16, TW], i16, tag="jd")
                    nc.sync.dma_start(
                        out=jd[:].rearrange("q (tl s) -> tl q s", tl=8),
                        in_=idx16[:, cs:cs + TCH])
                    jda = jd[:]
                    rep_src = bass.AP(tensor=jda.tensor, offset=jda.offset,
                                      ap=[[0, 8]] + list(jda.ap))
                    jw = jwp.tile([P, TW], i16, tag="jw")
                    nc.sync.dma_start(out=jw[:], in_=rep_src)
                    # permute t'' = (tl-major) -> t = 8s+tl (gather stream order)
                    jt = jtp.tile([P, TW], i16, tag="jt")
                    nc.vector.tensor_copy(
                        out=jt[:].rearrange("p (s tl) -> p tl s", tl=8),
                        in_=jw[:].rearrange("p (tl s) -> p tl s", s=TCH))

                    G = gatp.tile([P, TCH, 128], bf16, tag="G")
                    if gather:
                        gsl = GNI // P       # slots per gather call (8)
                        gw = GNI // 16       # idx cols per gather call (64)
                        for k in range(gcalls):
                            nc.gpsimd.dma_gather(
                                out_ap=G[:, k * gsl:(k + 1) * gsl, :],
                                in_ap=qtab,
                                idxs_ap=jt[:, k * gw:(k + 1) * gw],
                                num_idxs=GNI, num_idxs_reg=GNI, elem_size=128,
                                queue_num=k % NQUEUES)
                    else:
                        nc.vector.memset(G[:], 0.25)

                    Gv = G.rearrange("p t (r e) -> p t r e", r=2)
                    m2v = mb2[:, cs:cs + TCH, None, None]
                    m1v = mb1[:, cs:cs + TCH, None, None]
                    m0v = mb0[:, cs:cs + TCH, None, None]

                    if INPLACE_TREE:
                        # in-place select tree: conditional shifted copies onto
                        # G itself (reads run 16/8/4 columns ahead of writes;
                        # the DVE read stage pipelines well ahead of writes).
                        nc.vector.copy_predicated(
                            out=Gv[:, :, :, 0:20].bitcast(f32),
                            mask=m2v.to_broadcast([P, TCH, 2, 10]),
                            data=Gv[:, :, :, 16:36].bitcast(f32))
                        nc.vector.copy_predicated(
                            out=Gv[:, :, :, 0:12].bitcast(f32),
                            mask=m1v.to_broadcast([P, TCH, 2, 6]),
                            data=Gv[:, :, :, 8:20].bitcast(f32))
                        nc.vector.copy_predicated(
                            out=Gv[:, :, :, 0:8].bitcast(f32),
                            mask=m0v.to_broadcast([P, TCH, 2, 4]),
                            data=Gv[:, :, :, 4:12].bitcast(f32))
                        W3t = Gv
                    else:
                        W1t = shiftp.tile([P, TCH, 2, 20], bf16, tag="W1")
                        nc.vector.tensor_copy(out=W1t[:], in_=Gv[:, :, :, 0:20])
                        nc.vector.copy_predicated(
                            out=W1t[:].bitcast(f32),
                            mask=m2v.to_broadcast([P, TCH, 2, 10]),
                            data=Gv[:, :, :, 16:36].bitcast(f32))
                        W2t = shiftp.tile([P, TCH, 2, 12], bf16, tag="W2")
                        nc.vector.tensor_copy(out=W2t[:], in_=W1t[:, :, :, 0:12])
                        nc.vector.copy_predicated(
                            out=W2t[:].bitcast(f32),
                            mask=m1v.to_broadcast([P, TCH, 2, 6]),
                            data=W1t[:, :, :, 8:20].bitcast(f32))
                        W3t = shiftp.tile([P, TCH, 2, 8], bf16, tag="W3")
                        nc.vector.tensor_copy(out=W3t[:], in_=W2t[:, :, :, 0:8])
                        nc.vector.copy_predicated(
                            out=W3t[:].bitcast(f32),
                            mask=m0v.to_broadcast([P, TCH, 2, 4]),
                            data=W2t[:, :, :, 4:12].bitcast(f32))
                        W3t = W3t[:].rearrange("p t r e -> p t r e")

                    # lerp x then y -> staging features
                    wxv = wx[:, cs:cs + TCH, None, None].to_broadcast([P, TCH, 2, 4])
                    wyv = wy[:, cs:cs + TCH, None].to_broadcast([P, TCH, 4])
                    d = shiftp.tile([P, TCH, 2, 4], bf16, tag="d")
                    nc.vector.tensor_tensor(out=d[:], in0=W3t[:, :, :, 4:8],
                                            in1=W3t[:, :, :, 0:4], op=OP.subtract)
                    nc.vector.tensor_tensor(out=d[:], in0=d[:], in1=wxv, op=OP.mult)
                    nc.vector.tensor_tensor(out=d[:], in0=W3t[:, :, :, 0:4],
                                            in1=d[:], op=OP.add)
                    e = shiftp.tile([P, TCH, 4], bf16, tag="e")
                    nc.vector.tensor_tensor(out=e[:], in0=d[:, :, 1, :],
                                            in1=d[:, :, 0, :], op=OP.subtract)
                    nc.vector.tensor_tensor(out=e[:], in0=e[:], in1=wyv, op=OP.mult)
                    nc.vector.tensor_tensor(out=stg[:, cs:cs + TCH, 3:7],
                                            in0=d[:, :, 0, :], in1=e[:], op=OP.add)

                if not mlp:
                    ost = ostagep.tile([P, s_tile, 3], f32, tag="ost")
                    nc.scalar.activation(out=ost[:], in_=stg[:, :, 3:6],
                                         func=AF.Copy, bias=0.0, scale=1.0)
                    nc.sync.dma_start(out=yv[:, sl0:sl0 + s_tile, :], in_=ost[:])
                    continue

                # ======== MLP for the PREVIOUS tile (one-tile software
                # pipeline: keeps the next tile's coord/idx/gather chain off
                # the back of this tile's MLP in the Act queue) ========
                if pending_mlp is not None:
                    run_mlp(*pending_mlp)
                pending_mlp = (stg, sl0)

            if pending_mlp is not None:
                run_mlp(*pending_mlp)

    nc.compile()
    return nc



_PROGRAM_CACHE = {}


def _get_program(n_slots, s_tile):
    key = (n_slots, s_tile)
    if key not in _PROGRAM_CACHE:
        _PROGRAM_CACHE[key] = build_program(n_slots, s_tile)
    return _PROGRAM_CACHE[key]


def make_in_maps(x_full, consts, n_slots=M_SLOTS, n_cores=N_CORES):
    n_pad = P * n_slots
    per = x_full.shape[0] // n_cores
    in_maps = []
    for c in range(n_cores):
        xpad = np.zeros((n_pad, 3), np.float32)
        xpad[:per] = x_full[c * per:(c + 1) * per]
        in_maps.append({"x": xpad, **{k: np.ascontiguousarray(v)
                                      for k, v in consts.items()}})
    return in_maps


def kernel(**inputs):
    from concourse import bass_utils
    from concourse.bass_interp import get_hw_module

    x = np.asarray(inputs["x"], dtype=np.float32)
    fm = np.asarray(inputs["featuremap"], dtype=np.float32)
    Ws = [np.asarray(inputs[f"W{i}"], dtype=np.float32) for i in range(1, 6)]
    bs = [np.asarray(inputs[f"b{i}"], dtype=np.float32) for i in range(1, 6)]

    consts = _build_host_constants(fm, Ws, bs)
    n = x.shape[0]
    assert n == N_TOTAL, n
    per = n // N_CORES

    nc = _get_program(M_SLOTS, S)
    old_m = nc.m
    nc.m = get_hw_module(nc.m)
    try:
        in_maps = make_in_maps(x, consts)
        res = bass_utils.run_bass_kernel_spmd(nc, in_maps,
                                              core_ids=list(range(N_CORES)))
    finally:
        nc.m = old_m
    outs = [r["y"][:per] for r in res.results]
    return np.concatenate(outs, axis=0).astype(np.float32)


if __name__ == "__main__":
    build_program(256, 128)
    print("small program built OK")
